# revision 8
# baseline (speedup 1.0000x reference)
"""TRN2 Bass kernel for nn_LocalAttention (B=4, T=2048, C=1024, window=16).

Sharding: 8 cores = (batch b, row-half h). Each core projects K^T/V for its
OWN T-half only; the halves are exchanged with the sibling core via an
intra-pair AllGather (no duplicated projection work). Attention + out-proj
run on the core's own 1024 rows (two 512-row chunks; h=0 gets global chunks
{0,3}, h=1 gets {1,2}; slot 0 = denser chunk).

All matmuls run in fp32r (TF32-like, ~1.5e-4 rel err, 4x fp32 speed). Raw
fp32 bytes are declared as fp32r at the DRAM boundary - the PE rounds
internally (validated: identical error to explicit cast-DMA).

Orientation trick: host passes X^T and W^T so every matmul is natural:
  K^T = (Wk^T)^T @ X_half^T   [C, 1024]  -> AllGather -> [2][C, 1024]
  V   = (X_half^T)^T @ Wv^T   [1024, C]  -> AllGather
  Q^T = (Wq^T)^T @ X_own^T    [C, 1024]  (SBUF resident)
  S^T = (K^T_blk)^T @ Q^T_chunk  -> [keys, rows]; softmax-over-keys is a
        partition reduction done by a ones-vector matmul, and E^T feeds
  Y^T = V_blk^T @ E^T            [C, rows]
  Z^T = (Wo^T)^T @ Y^T           [C, rows]

Sparsity: mask keeps j >= i - 16 (reverse-causal), so each 512-row chunk's
kept key-block set is a SUFFIX {b..15}; processing key blocks in descending
order (position p -> block 15-p) makes every kept set a static PREFIX.
Chunk slot 0 runs 16 positions, slot 1 runs 9 - uniform across cores; the
data-driven is_ge mask zeroes over-included blocks. Mask applied
multiplicatively post-exp (scores are O(6), no overflow without max-sub).
"""
import numpy as np

import concourse.bass as bass
import concourse.mybir as mybir
import concourse.tile as tile
from concourse import bacc
from concourse import bass_utils

N_CORES = 8
B, T, C = 4, 2048, 1024
WINDOW = 16
TOWN = T // 2          # own rows per core
THALF = T // 2         # K/V half computed per core
CHUNK = 512            # rows per processing chunk
NCHUNK = TOWN // CHUNK  # 2
CI = C // 128          # 8 contraction blocks
CO = C // 128          # 8 output blocks
KB = T // 128          # 16 key blocks (global)
KBH = THALF // 128     # 8 key blocks per half
SLOT_KBS = (16, 9)     # key-block positions per chunk slot (descending order)
F32 = mybir.dt.float32
F32R = mybir.dt.float32r

_NC_CACHE = {}


def build():
    if "nc" in _NC_CACHE:
        return _NC_CACHE["nc"]
    nc = bacc.Bacc("TRN2", target_bir_lowering=False, debug=False,
                   num_devices=N_CORES)
    xtm = nc.dram_tensor("xtm", [C, THALF], F32R, kind="ExternalInput").ap()
    xtq = nc.dram_tensor("xtq", [C, TOWN], F32R, kind="ExternalInput").ap()
    wqt = nc.dram_tensor("wqt", [C, C], F32R, kind="ExternalInput").ap()
    wkt = nc.dram_tensor("wkt", [C, C], F32R, kind="ExternalInput").ap()
    wvt = nc.dram_tensor("wvt", [C, C], F32R, kind="ExternalInput").ap()
    wot = nc.dram_tensor("wot", [C, C], F32R, kind="ExternalInput").ap()
    keyidx16 = nc.dram_tensor("keyidx16", [128, KB], F32, kind="ExternalInput").ap()
    rowidxb = nc.dram_tensor("rowidxb", [128, TOWN], F32, kind="ExternalInput").ap()
    zt = nc.dram_tensor("zt", [C, TOWN], F32, kind="ExternalOutput").ap()

    xtm3 = xtm.rearrange("(ko ki) t -> ki ko t", ki=128)
    xtq3 = xtq.rearrange("(ko ki) t -> ki ko t", ki=128)
    w3 = {w.tensor.name: w.rearrange("(ko ki) c -> ki ko c", ki=128)
          for w in (wqt, wkt, wvt, wot)}

    inv_sqrt_c = float(1.0 / np.sqrt(C))

    with tile.TileContext(nc) as tc:
        with tc.tile_pool(name="res", bufs=1) as res, \
             tc.tile_pool(name="dram", bufs=1, space="DRAM") as dram:
            # kvh[0] = own K^T half [ki, ko, t_local]; kvh[1] = own V half
            # [ki(t), tb, c]. AllGather over the pair -> kvg[peer][which].
            kvh_d = dram.tile([2, 128, KBH, THALF], F32R)
            kvg_d = dram.tile([2, 2, 128, KBH, THALF], F32R)
            qt_sb = res.tile([128, CI, TOWN], F32R, tag="qt")  # Q^T resident
            wo_sb = res.tile([128, CI, C], F32R, tag="wo")
            ki16_sb = res.tile([128, KB], F32, tag="ki16")
            nc.gpsimd.dma_start(ki16_sb[:], keyidx16[:])
            ones_row_f32 = res.tile([1, 128], F32, tag="onesrf")
            nc.vector.memset(ones_row_f32[:], 1.0)
            ones_1x128 = res.tile([1, 128], F32R, tag="o1")
            nc.vector.tensor_copy(ones_1x128[:], ones_row_f32[:])
            ones_col_f32 = res.tile([128, 1], F32, tag="onescf")
            nc.vector.memset(ones_col_f32[:], 1.0)
            ones_128x1 = res.tile([128, 1], F32R, tag="o2")
            nc.vector.tensor_copy(ones_128x1[:], ones_col_f32[:])

            # ================= Phase A: projections =========================
            with tc.tile_pool(name="wts", bufs=1) as wts, \
                 tc.tile_pool(name="xa", bufs=2) as xa, \
                 tc.tile_pool(name="stg", bufs=3) as stg, \
                 tc.tile_pool(name="ps_k", bufs=3, space="PSUM") as ps_k, \
                 tc.tile_pool(name="ps_v", bufs=2, space="PSUM") as ps_v, \
                 tc.tile_pool(name="ps_q", bufs=2, space="PSUM") as ps_q:
                wk_sb = wts.tile([128, CI, C], F32R, tag="wk")
                wv_sb = wts.tile([128, CI, C], F32R, tag="wv")
                wq_sb = wts.tile([128, CI, C], F32R, tag="wq")
                # first xt chunk + first weight column before the rest
                xt_sb0 = xa.tile([128, CI, CHUNK], F32R, tag="xa")
                nc.sync.dma_start(xt_sb0[:], xtm3[:, :, 0:CHUNK])
                for co in range(CO):  # per-column loads: co=0 unblocks MMs
                    nc.sync.dma_start(wk_sb[:, :, co * 128:(co + 1) * 128],
                                      w3["wkt"][:, :, co * 128:(co + 1) * 128])
                for ci in range(CI):
                    nc.scalar.dma_start(wv_sb[:, ci, :], w3["wvt"][:, ci, :])
                for ci in range(CI):
                    nc.scalar.dma_start(wq_sb[:, ci, :], w3["wqt"][:, ci, :])

                for tch in range(THALF // CHUNK):  # 2 chunks of own half
                    if tch == 0:
                        xt_sb = xt_sb0
                    else:
                        xt_sb = xa.tile([128, CI, CHUNK], F32R, tag="xa")
                        nc.sync.dma_start(
                            xt_sb[:], xtm3[:, :, tch * CHUNK:(tch + 1) * CHUNK])
                    # K^T half [cout, t_local]
                    for co in range(CO):
                        kps = ps_k.tile([128, CHUNK], F32, tag="kps")
                        for ci in range(CI):
                            nc.tensor.matmul(
                                kps[:], wk_sb[:, ci, co * 128:(co + 1) * 128],
                                xt_sb[:, ci, :], start=(ci == 0), stop=(ci == CI - 1))
                        kstage = stg.tile([128, CHUNK], F32R, tag="kstage")
                        nc.vector.tensor_copy(kstage[:], kps[:])
                        nc.sync.dma_start(
                            kvh_d[0, :, co, tch * CHUNK:(tch + 1) * CHUNK],
                            kstage[:])
                    # V half [t_local, cout]
                    for tb in range(CHUNK // 128):
                        for half in range(2):
                            vps = ps_v.tile([128, 512], F32, tag="vps")
                            for ci in range(CI):
                                nc.tensor.matmul(
                                    vps[:], xt_sb[:, ci, tb * 128:(tb + 1) * 128],
                                    wv_sb[:, ci, half * 512:(half + 1) * 512],
                                    start=(ci == 0), stop=(ci == CI - 1))
                            vstage = stg.tile([128, 512], F32R, tag="vstage")
                            nc.vector.tensor_copy(vstage[:], vps[:])
                            nc.scalar.dma_start(
                                kvh_d[1, :, tch * (CHUNK // 128) + tb,
                                      half * 512:(half + 1) * 512], vstage[:])

                # exchange halves with the sibling core (overlaps Q proj)
                nc.gpsimd.collective_compute(
                    "AllGather", mybir.AluOpType.bypass,
                    replica_groups=[[2 * p, 2 * p + 1] for p in range(N_CORES // 2)],
                    ins=[kvh_d.opt()], outs=[kvg_d.opt()])

                for qch in range(TOWN // CHUNK):
                    xq_sb = xa.tile([128, CI, CHUNK], F32R, tag="xa")
                    nc.sync.dma_start(
                        xq_sb[:], xtq3[:, :, qch * CHUNK:(qch + 1) * CHUNK])
                    for co in range(CO):
                        qps = ps_q.tile([128, CHUNK], F32, tag="qps")
                        for ci in range(CI):
                            nc.tensor.matmul(
                                qps[:], wq_sb[:, ci, co * 128:(co + 1) * 128],
                                xq_sb[:, ci, :], start=(ci == 0), stop=(ci == CI - 1))
                        nc.vector.tensor_copy(
                            qt_sb[:, co, qch * CHUNK:(qch + 1) * CHUNK], qps[:])

            # wo load late on the scalar queue (used only at Z, end of B)
            for ci in range(CI):
                nc.scalar.dma_start(wo_sb[:, ci, :], w3["wot"][:, ci, :])

            # ================= Phase B: attention + out-proj ================
            with tc.tile_pool(name="et", bufs=1) as etp, \
                 tc.tile_pool(name="ktb", bufs=4) as ktb_p, \
                 tc.tile_pool(name="vco", bufs=3) as vsp, \
                 tc.tile_pool(name="ysb", bufs=2) as ysb_p, \
                 tc.tile_pool(name="wb", bufs=2) as wb, \
                 tc.tile_pool(name="zst", bufs=3) as zstp, \
                 tc.tile_pool(name="ps_s", bufs=3, space="PSUM") as ps_s, \
                 tc.tile_pool(name="ps_sh", bufs=1, space="PSUM") as ps_sh, \
                 tc.tile_pool(name="ps_y", bufs=2, space="PSUM") as ps_y, \
                 tc.tile_pool(name="ps_z", bufs=2, space="PSUM") as ps_z:
                for ch in range(NCHUNK):
                    nkb = SLOT_KBS[ch]
                    rsl = slice(ch * CHUNK, (ch + 1) * CHUNK)
                    ri_b = wb.tile([128, CHUNK], F32, tag="rib")
                    nc.sync.dma_start(ri_b[:], rowidxb[:, rsl])

                    et = etp.tile([128, KB, CHUNK], F32R, tag="et")
                    # --- sweep 1a: scores + exp + mask (descending kb) ---
                    for p in range(nkb):
                        kb = KB - 1 - p
                        peer, lb = divmod(kb, KBH)
                        kt_b = ktb_p.tile([128, CI, 128], F32R, tag="ktb")
                        nc.scalar.dma_start(
                            kt_b[:], kvg_d[peer, 0, :, :, lb * 128:(lb + 1) * 128])
                        sps = ps_s.tile([128, CHUNK], F32, tag="sps")
                        for ci in range(CI):
                            nc.tensor.matmul(
                                sps[:], kt_b[:, ci, :], qt_sb[:, ci, rsl],
                                start=(ci == 0), stop=(ci == CI - 1))
                        nc.scalar.activation(et[:, p, :], sps[:],
                                             mybir.ActivationFunctionType.Exp,
                                             scale=inv_sqrt_c)
                        mask = wb.tile([128, CHUNK], F32, tag="mask")
                        nc.vector.tensor_tensor(
                            mask[:], ki16_sb[:, kb:kb + 1].to_broadcast((128, CHUNK)),
                            ri_b[:], mybir.AluOpType.is_ge)
                        nc.vector.tensor_tensor(et[:, p, :], et[:, p, :], mask[:],
                                                mybir.AluOpType.mult)
                    # --- sweep 1b: key-sums via ones matmul ---
                    sums_ps = ps_sh.tile([1, CHUNK], F32, tag="shared")
                    for p in range(nkb):
                        nc.tensor.matmul(sums_ps[:], ones_128x1[:], et[:, p, :],
                                         start=(p == 0), stop=(p == nkb - 1))
                    recip = wb.tile([1, CHUNK], F32R, tag="recip")
                    with nc.allow_low_precision(reason="fp32r normalizer broadcast"):
                        nc.vector.reciprocal(recip[:], sums_ps[:])
                    rb_ps = ps_sh.tile([128, CHUNK], F32, tag="shared")
                    nc.tensor.matmul(rb_ps[:], ones_1x128[:], recip[:],
                                     start=True, stop=True)
                    rb_sb = wb.tile([128, CHUNK], F32, tag="rbsb")
                    nc.vector.tensor_copy(rb_sb[:], rb_ps[:])

                    # --- sweep 2: Y^T = V^T @ E^T per cout block ---
                    # v_co indexed by GLOBAL kb; loads split per peer segment
                    y_sb = ysb_p.tile([128, CO, CHUNK], F32R, tag="ysb")
                    for co in range(CO):
                        v_co = vsp.tile([128, KB, 128], F32R, tag="vco")
                        lo_kb = KB - nkb  # lowest global kb used
                        # peer 1 segment: kb 8..15 -> v_co[:, 8:16]
                        nc.sync.dma_start(
                            v_co[:, max(lo_kb, KBH):KB, :],
                            kvg_d[1, 1, :, max(lo_kb - KBH, 0):, 
                                  co * 128:(co + 1) * 128])
                        if lo_kb < KBH:  # peer 0 segment: kb lo..7
                            nc.sync.dma_start(
                                v_co[:, lo_kb:KBH, :],
                                kvg_d[0, 1, :, lo_kb:, co * 128:(co + 1) * 128])
                        yps = ps_y.tile([128, CHUNK], F32, tag="yps")
                        for p in range(nkb):
                            kb = KB - 1 - p
                            nc.tensor.matmul(yps[:], v_co[:, kb, :], et[:, p, :],
                                             start=(p == 0), stop=(p == nkb - 1))
                        nc.vector.tensor_copy(y_sb[:, co, :], yps[:])

                    # --- out-proj + normalize ---
                    for co in range(CO):
                        zps = ps_z.tile([128, CHUNK], F32, tag="zps")
                        for ci in range(CI):
                            nc.tensor.matmul(
                                zps[:], wo_sb[:, ci, co * 128:(co + 1) * 128],
                                y_sb[:, ci, :], start=(ci == 0), stop=(ci == CI - 1))
                        zst = zstp.tile([128, CHUNK], F32, tag="zst")
                        nc.vector.tensor_tensor(zst[:], zps[:], rb_sb[:],
                                                mybir.AluOpType.mult)
                        nc.sync.dma_start(zt[co * 128:(co + 1) * 128, rsl], zst[:])
    nc.compile()
    _NC_CACHE["nc"] = nc
    return nc


def make_in_maps(inputs):
    x = np.asarray(inputs["x"], dtype=np.float32)
    for bname in ("bq", "bk", "bv", "bo"):
        bval = np.asarray(inputs[bname])
        assert np.all(bval == 0.0), f"{bname} nonzero: unsupported fast path"
    wqt = np.ascontiguousarray(np.asarray(inputs["Wq"], np.float32).T)
    wkt = np.ascontiguousarray(np.asarray(inputs["Wk"], np.float32).T)
    wvt = np.ascontiguousarray(np.asarray(inputs["Wv"], np.float32).T)
    wot = np.ascontiguousarray(np.asarray(inputs["Wo"], np.float32).T)
    keyidx16 = (np.arange(T, dtype=np.float32).reshape(KB, 128).T + WINDOW
                ).copy()  # [128, KB]
    chunk_map = {0: (0, 3), 1: (1, 2)}  # slot 0 = denser chunk
    in_maps = []
    for core in range(N_CORES):
        b, h = divmod(core, 2)
        xt_b = np.ascontiguousarray(x[b].T)  # [C, T]
        ch0, ch1 = chunk_map[h]
        xtm = xt_b[:, h * THALF:(h + 1) * THALF]  # own K/V half
        xtq = np.concatenate(
            [xt_b[:, ch0 * CHUNK:(ch0 + 1) * CHUNK],
             xt_b[:, ch1 * CHUNK:(ch1 + 1) * CHUNK]], axis=1)
        rowidx = np.concatenate(
            [np.arange(ch0 * CHUNK, (ch0 + 1) * CHUNK, dtype=np.float32),
             np.arange(ch1 * CHUNK, (ch1 + 1) * CHUNK, dtype=np.float32)])
        rowidxb = np.ascontiguousarray(
            np.broadcast_to(rowidx[None, :], (128, TOWN)))
        in_maps.append({
            "xtm": np.ascontiguousarray(xtm), "xtq": np.ascontiguousarray(xtq),
            "wqt": wqt, "wkt": wkt, "wvt": wvt, "wot": wot,
            "keyidx16": keyidx16, "rowidxb": rowidxb,
        })
    return in_maps


def gather_output(results, dtype):
    out = np.empty((B, T, C), dtype=dtype)
    chunk_map = {0: (0, 3), 1: (1, 2)}
    for core in range(N_CORES):
        b, h = divmod(core, 2)
        y = results[core]["zt"].T  # [TOWN rows, C]
        ch0, ch1 = chunk_map[h]
        out[b, ch0 * CHUNK:(ch0 + 1) * CHUNK] = y[:CHUNK]
        out[b, ch1 * CHUNK:(ch1 + 1) * CHUNK] = y[CHUNK:]
    return out


def kernel(**inputs):
    nc = build()
    in_maps = make_in_maps(inputs)
    res = bass_utils.run_bass_kernel_spmd(nc, in_maps,
                                          core_ids=list(range(N_CORES)))
    return gather_output(res.results, np.asarray(inputs["x"]).dtype)


# revision 9
# speedup vs baseline: 1.4607x; 1.4607x over previous
"""TRN2 Bass kernel for nn_LocalAttention (B=4, T=2048, C=1024, window=16).

Sharding: 8 cores = (batch b, row-half h). Each core computes K^T/V for its
whole batch (duplicated across the 2 cores of a batch) and attention +
projections for its own 1024 rows (two 512-row chunks; h=0 gets global
chunks {0,3}, h=1 gets {1,2}; slot 0 = denser chunk).

All matmuls run in fp32r (TF32-like, ~1.5e-4 rel err, 4x fp32 speed). Raw
fp32 bytes are declared as fp32r at the DRAM boundary - the PE rounds
internally (validated: identical error to explicit cast-DMA).

Orientation trick: host passes X^T and W^T so every matmul is natural:
  K^T = (Wk^T)^T @ X^T        [C, T]     (DRAM scratch)
  V   = (X^T)^T @ Wv^T        [T, C]     (DRAM scratch)
  Q^T = (Wq^T)^T @ X_own^T    [C, 1024]  (SBUF resident)
  S^T = (K^T_blk)^T @ Q^T_chunk  -> [keys, rows]; softmax-over-keys is a
        partition reduction done by a ones-vector matmul, and E^T feeds
  Y^T = V_blk^T @ E^T            [C, rows]
  Z^T = (Wo^T)^T @ Y^T           [C, rows]

Sparsity: mask keeps j >= i - 16 (reverse-causal), so each 512-row chunk's
kept key-block set is a SUFFIX {b..15}; processing key blocks in descending
order (position p -> block 15-p) makes every kept set a static PREFIX.
Chunk slot 0 runs 16 positions, slot 1 runs 9 - uniform across cores, the
data-driven is_ge mask zeroes over-included blocks. Mask applied
multiplicatively post-exp (scores are O(6), no overflow without max-sub).
"""
import numpy as np

import concourse.bass as bass
import concourse.mybir as mybir
import concourse.tile as tile
from concourse import bacc
from concourse import bass_utils

N_CORES = 8
B, T, C = 4, 2048, 1024
WINDOW = 16
TOWN = T // 2          # own rows per core
CHUNK = 512            # rows per processing chunk
NCHUNK = TOWN // CHUNK  # 2
CI = C // 128          # 8 contraction blocks
CO = C // 128          # 8 output blocks
KB = T // 128          # 16 key blocks
TCH = T // CHUNK       # 4 t-chunks in phase A
SLOT_KBS = (16, 9)     # key-block positions per chunk slot (descending order)
F32 = mybir.dt.float32
F32R = mybir.dt.float32r

_NC_CACHE = {}


def build():
    if "nc" in _NC_CACHE:
        return _NC_CACHE["nc"]
    nc = bacc.Bacc("TRN2", target_bir_lowering=False, debug=False,
                   num_devices=N_CORES)
    xt = nc.dram_tensor("xt", [C, T], F32R, kind="ExternalInput").ap()
    xtq = nc.dram_tensor("xtq", [C, TOWN], F32R, kind="ExternalInput").ap()
    wqt = nc.dram_tensor("wqt", [C, C], F32R, kind="ExternalInput").ap()
    wkt = nc.dram_tensor("wkt", [C, C], F32R, kind="ExternalInput").ap()
    wvt = nc.dram_tensor("wvt", [C, C], F32R, kind="ExternalInput").ap()
    wot = nc.dram_tensor("wot", [C, C], F32R, kind="ExternalInput").ap()
    keyidx16 = nc.dram_tensor("keyidx16", [128, KB], F32, kind="ExternalInput").ap()
    rowidxb = nc.dram_tensor("rowidxb", [128, TOWN], F32, kind="ExternalInput").ap()
    zt = nc.dram_tensor("zt", [C, TOWN], F32, kind="ExternalOutput").ap()

    xt3 = xt.rearrange("(ko ki) t -> ki ko t", ki=128)
    xtq3 = xtq.rearrange("(ko ki) t -> ki ko t", ki=128)
    w3 = {w.tensor.name: w.rearrange("(ko ki) c -> ki ko c", ki=128)
          for w in (wqt, wkt, wvt, wot)}

    inv_sqrt_c = float(1.0 / np.sqrt(C))

    with tile.TileContext(nc) as tc:
        with tc.tile_pool(name="res", bufs=1) as res, \
             tc.tile_pool(name="dram", bufs=1, space="DRAM") as dram:
            kt_d = dram.tile([128, CI, T], F32R)      # K^T  [ki, ko, t]
            v_d = dram.tile([128, KB, C], F32R)       # V    [ki, ko, c]
            qt_sb = res.tile([128, CI, TOWN], F32R, tag="qt")  # Q^T resident
            wo_sb = res.tile([128, CI, C], F32R, tag="wo")
            ki16_sb = res.tile([128, KB], F32, tag="ki16")
            nc.gpsimd.dma_start(ki16_sb[:], keyidx16[:])
            ones_row_f32 = res.tile([1, 128], F32, tag="onesrf")
            nc.vector.memset(ones_row_f32[:], 1.0)
            ones_1x128 = res.tile([1, 128], F32R, tag="o1")
            nc.vector.tensor_copy(ones_1x128[:], ones_row_f32[:])
            ones_col_f32 = res.tile([128, 1], F32, tag="onescf")
            nc.vector.memset(ones_col_f32[:], 1.0)
            ones_128x1 = res.tile([128, 1], F32R, tag="o2")
            nc.vector.tensor_copy(ones_128x1[:], ones_col_f32[:])

            # ================= Phase A: projections =========================
            with tc.tile_pool(name="wts", bufs=1) as wts, \
                 tc.tile_pool(name="xa", bufs=2) as xa, \
                 tc.tile_pool(name="stg", bufs=3) as stg, \
                 tc.tile_pool(name="ps_k", bufs=3, space="PSUM") as ps_k, \
                 tc.tile_pool(name="ps_v", bufs=2, space="PSUM") as ps_v, \
                 tc.tile_pool(name="ps_q", bufs=2, space="PSUM") as ps_q:
                wk_sb = wts.tile([128, CI, C], F32R, tag="wk")
                wv_sb = wts.tile([128, CI, C], F32R, tag="wv")
                wq_sb = wts.tile([128, CI, C], F32R, tag="wq")
                # first xt chunk before anything else on the sync queue
                xt_sbs = []
                xt_sb0 = xa.tile([128, CI, CHUNK], F32R, tag="xa")
                nc.sync.dma_start(xt_sb0[:], xt3[:, :, (TCH - 1) * CHUNK:TCH * CHUNK])
                for co in range(CO):  # per-column loads: co=0 unblocks MMs
                    nc.sync.dma_start(wk_sb[:, :, co * 128:(co + 1) * 128],
                                      w3["wkt"][:, :, co * 128:(co + 1) * 128])
                for ci in range(CI):
                    nc.scalar.dma_start(wv_sb[:, ci, :], w3["wvt"][:, ci, :])
                for ci in range(CI):
                    nc.scalar.dma_start(wq_sb[:, ci, :], w3["wqt"][:, ci, :])

                for tch in reversed(range(TCH)):
                    if tch == TCH - 1:
                        xt_sb = xt_sb0
                    else:
                        xt_sb = xa.tile([128, CI, CHUNK], F32R, tag="xa")
                        nc.sync.dma_start(
                            xt_sb[:], xt3[:, :, tch * CHUNK:(tch + 1) * CHUNK])
                    # K^T [cout, t]
                    for co in range(CO):
                        kps = ps_k.tile([128, CHUNK], F32, tag="kps")
                        for ci in range(CI):
                            nc.tensor.matmul(
                                kps[:], wk_sb[:, ci, co * 128:(co + 1) * 128],
                                xt_sb[:, ci, :], start=(ci == 0), stop=(ci == CI - 1))
                        kstage = stg.tile([128, CHUNK], F32R, tag="kstage")
                        nc.vector.tensor_copy(kstage[:], kps[:])
                        nc.sync.dma_start(
                            kt_d[:, co, tch * CHUNK:(tch + 1) * CHUNK], kstage[:])
                    # V [t, cout]
                    for tb in range(CHUNK // 128):
                        for half in range(2):
                            vps = ps_v.tile([128, 512], F32, tag="vps")
                            for ci in range(CI):
                                nc.tensor.matmul(
                                    vps[:], xt_sb[:, ci, tb * 128:(tb + 1) * 128],
                                    wv_sb[:, ci, half * 512:(half + 1) * 512],
                                    start=(ci == 0), stop=(ci == CI - 1))
                            vstage = stg.tile([128, 512], F32R, tag="vstage")
                            nc.vector.tensor_copy(vstage[:], vps[:])
                            nc.scalar.dma_start(
                                v_d[:, tch * (CHUNK // 128) + tb,
                                    half * 512:(half + 1) * 512], vstage[:])

                for qch in range(TOWN // CHUNK):
                    xq_sb = xa.tile([128, CI, CHUNK], F32R, tag="xa")
                    nc.sync.dma_start(
                        xq_sb[:], xtq3[:, :, qch * CHUNK:(qch + 1) * CHUNK])
                    for co in range(CO):
                        qps = ps_q.tile([128, CHUNK], F32, tag="qps")
                        for ci in range(CI):
                            nc.tensor.matmul(
                                qps[:], wq_sb[:, ci, co * 128:(co + 1) * 128],
                                xq_sb[:, ci, :], start=(ci == 0), stop=(ci == CI - 1))
                        nc.vector.tensor_copy(
                            qt_sb[:, co, qch * CHUNK:(qch + 1) * CHUNK], qps[:])

            # wo load late on the scalar queue (used only at Z, end of B)
            for ci in range(CI):
                nc.scalar.dma_start(wo_sb[:, ci, :], w3["wot"][:, ci, :])

            # ================= Phase B: attention + out-proj ================
            with tc.tile_pool(name="et", bufs=1) as etp, \
                 tc.tile_pool(name="ktb", bufs=4) as ktb_p, \
                 tc.tile_pool(name="vco", bufs=3) as vsp, \
                 tc.tile_pool(name="ysb", bufs=2) as ysb_p, \
                 tc.tile_pool(name="wb", bufs=2) as wb, \
                 tc.tile_pool(name="zst", bufs=3) as zstp, \
                 tc.tile_pool(name="ps_s", bufs=3, space="PSUM") as ps_s, \
                 tc.tile_pool(name="ps_sh", bufs=1, space="PSUM") as ps_sh, \
                 tc.tile_pool(name="ps_y", bufs=2, space="PSUM") as ps_y, \
                 tc.tile_pool(name="ps_z", bufs=2, space="PSUM") as ps_z:
                for ch in range(NCHUNK):
                    nkb = SLOT_KBS[ch]
                    rsl = slice(ch * CHUNK, (ch + 1) * CHUNK)
                    ri_b = wb.tile([128, CHUNK], F32, tag="rib")
                    nc.sync.dma_start(ri_b[:], rowidxb[:, rsl])

                    et = etp.tile([128, KB, CHUNK], F32R, tag="et")
                    # --- sweep 1a: scores + exp + mask (descending kb) ---
                    for p in range(nkb):
                        kb = KB - 1 - p
                        kt_b = ktb_p.tile([128, CI, 128], F32R, tag="ktb")
                        nc.scalar.dma_start(
                            kt_b[:], kt_d[:, :, kb * 128:(kb + 1) * 128])
                        sps = ps_s.tile([128, CHUNK], F32, tag="sps")
                        for ci in range(CI):
                            nc.tensor.matmul(
                                sps[:], kt_b[:, ci, :], qt_sb[:, ci, rsl],
                                start=(ci == 0), stop=(ci == CI - 1))
                        nc.scalar.activation(et[:, p, :], sps[:],
                                             mybir.ActivationFunctionType.Exp,
                                             scale=inv_sqrt_c)
                        mask = wb.tile([128, CHUNK], F32, tag="mask")
                        nc.vector.tensor_tensor(
                            mask[:], ki16_sb[:, kb:kb + 1].to_broadcast((128, CHUNK)),
                            ri_b[:], mybir.AluOpType.is_ge)
                        nc.vector.tensor_tensor(et[:, p, :], et[:, p, :], mask[:],
                                                mybir.AluOpType.mult)
                    # --- sweep 1b: key-sums via ones matmul ---
                    sums_ps = ps_sh.tile([1, CHUNK], F32, tag="shared")
                    for p in range(nkb):
                        nc.tensor.matmul(sums_ps[:], ones_128x1[:], et[:, p, :],
                                         start=(p == 0), stop=(p == nkb - 1))
                    recip = wb.tile([1, CHUNK], F32R, tag="recip")
                    with nc.allow_low_precision(reason="fp32r normalizer broadcast"):
                        nc.vector.reciprocal(recip[:], sums_ps[:])
                    rb_ps = ps_sh.tile([128, CHUNK], F32, tag="shared")
                    nc.tensor.matmul(rb_ps[:], ones_1x128[:], recip[:],
                                     start=True, stop=True)
                    rb_sb = wb.tile([128, CHUNK], F32, tag="rbsb")
                    nc.vector.tensor_copy(rb_sb[:], rb_ps[:])

                    # --- sweep 2: Y^T = V^T @ E^T per cout block ---
                    y_sb = ysb_p.tile([128, CO, CHUNK], F32R, tag="ysb")
                    for co in range(CO):
                        v_co = vsp.tile([128, KB, 128], F32R, tag="vco")
                        nc.sync.dma_start(
                            v_co[:, :nkb, :],
                            v_d[:, KB - nkb:, co * 128:(co + 1) * 128])
                        yps = ps_y.tile([128, CHUNK], F32, tag="yps")
                        for p in range(nkb):
                            nc.tensor.matmul(yps[:], v_co[:, nkb - 1 - p, :],
                                             et[:, p, :],
                                             start=(p == 0), stop=(p == nkb - 1))
                        nc.vector.tensor_copy(y_sb[:, co, :], yps[:])

                    # --- out-proj + normalize ---
                    for co in range(CO):
                        zps = ps_z.tile([128, CHUNK], F32, tag="zps")
                        for ci in range(CI):
                            nc.tensor.matmul(
                                zps[:], wo_sb[:, ci, co * 128:(co + 1) * 128],
                                y_sb[:, ci, :], start=(ci == 0), stop=(ci == CI - 1))
                        zst = zstp.tile([128, CHUNK], F32, tag="zst")
                        nc.vector.tensor_tensor(zst[:], zps[:], rb_sb[:],
                                                mybir.AluOpType.mult)
                        nc.sync.dma_start(zt[co * 128:(co + 1) * 128, rsl], zst[:])
    nc.compile()
    _NC_CACHE["nc"] = nc
    return nc


def make_in_maps(inputs):
    x = np.asarray(inputs["x"], dtype=np.float32)
    for bname in ("bq", "bk", "bv", "bo"):
        bval = np.asarray(inputs[bname])
        assert np.all(bval == 0.0), f"{bname} nonzero: unsupported fast path"
    wqt = np.ascontiguousarray(np.asarray(inputs["Wq"], np.float32).T)
    wkt = np.ascontiguousarray(np.asarray(inputs["Wk"], np.float32).T)
    wvt = np.ascontiguousarray(np.asarray(inputs["Wv"], np.float32).T)
    wot = np.ascontiguousarray(np.asarray(inputs["Wo"], np.float32).T)
    keyidx16 = (np.arange(T, dtype=np.float32).reshape(KB, 128).T + WINDOW
                ).copy()  # [128, KB]
    chunk_map = {0: (0, 3), 1: (1, 2)}  # slot 0 = denser chunk
    in_maps = []
    for core in range(N_CORES):
        b, h = divmod(core, 2)
        xt_b = np.ascontiguousarray(x[b].T)  # [C, T]
        ch0, ch1 = chunk_map[h]
        xtq = np.concatenate(
            [xt_b[:, ch0 * CHUNK:(ch0 + 1) * CHUNK],
             xt_b[:, ch1 * CHUNK:(ch1 + 1) * CHUNK]], axis=1)
        rowidx = np.concatenate(
            [np.arange(ch0 * CHUNK, (ch0 + 1) * CHUNK, dtype=np.float32),
             np.arange(ch1 * CHUNK, (ch1 + 1) * CHUNK, dtype=np.float32)])
        rowidxb = np.ascontiguousarray(
            np.broadcast_to(rowidx[None, :], (128, TOWN)))
        in_maps.append({
            "xt": xt_b, "xtq": np.ascontiguousarray(xtq),
            "wqt": wqt, "wkt": wkt, "wvt": wvt, "wot": wot,
            "keyidx16": keyidx16, "rowidxb": rowidxb,
        })
    return in_maps


def gather_output(results, dtype):
    out = np.empty((B, T, C), dtype=dtype)
    chunk_map = {0: (0, 3), 1: (1, 2)}
    for core in range(N_CORES):
        b, h = divmod(core, 2)
        y = results[core]["zt"].T  # [TOWN rows, C]
        ch0, ch1 = chunk_map[h]
        out[b, ch0 * CHUNK:(ch0 + 1) * CHUNK] = y[:CHUNK]
        out[b, ch1 * CHUNK:(ch1 + 1) * CHUNK] = y[CHUNK:]
    return out


def kernel(**inputs):
    nc = build()
    in_maps = make_in_maps(inputs)
    res = bass_utils.run_bass_kernel_spmd(nc, in_maps,
                                          core_ids=list(range(N_CORES)))
    return gather_output(res.results, np.asarray(inputs["x"]).dtype)
